# revision 16
# baseline (speedup 1.0000x reference)
"""Trainium2 Bass kernel for the BayesianBeliefNetwork block (8-core SPMD).

Math (see problem reference):
  h    = LayerNorm(x)*gamma + beta                          [B,S,H]
  ev   = sigmoid(mean_s(h @ W_ve.T + b_ve))                 [B,V]
  post = belief-prop(ev, parents, var_emb, cpt_emb)         [B,V]  (5 iters)
  out  = [h, post] @ W_out.T + b_out + x                    [B,S,H]

Sharding: data-parallel over the B*S = 8192 tokens; core c owns 1024 tokens
(batch b = c//2, sequence half c%2).  Parameters replicated.  The per-batch
evidence is completed with a pairwise AllReduce of the [V] partial logit
sums between the two cores sharing a batch.

Device layout: transposed - H on partitions, tokens on the free axis.
LayerNorm folds into the matmul epilogue:

  out^T[ho,t] = rstd_t * (W1g^T xbf)[ho,t]        W1g = W_out[:, :H]*gamma
              - r1[ho] * (mu_t*rstd_t)            r1  = W1g.sum(hin)
              + xbf^T[ho,t]                       (bf16 residual)
              + (b_out + W1@beta + W2@post)[ho]   (after belief prop)

Schedule: evidence path FIRST (logits + sum(x) + sum(x^2) ones-stationary
matmuls over all chunks, ~25us), so the tiny AllReduce issues early and
belief-prop + ccol complete while the dominant ~131us W1 matmul stream is
still running; each output chunk is evicted and DMA'd as soon as its
accumulation group (and ccol) is done, leaving a ~4us tail after the last
matmul.  PE warm-up matmuls trip the HAM clock gate before real work.

rstd = Rsqrt(var+eps) on ScalarE (a single activation table, also reused
by belief-prop's 1/||pe||).  The cosine's 1/denom parent normalization
cancels (scale-invariant) so belief prop needs no parent-count division;
1/||cpt|| is precomputed on the host.
"""

import numpy as np
import ml_dtypes

import concourse.bass as bass
import concourse.tile as tile
from concourse import bacc, mybir
from concourse.bass_utils import run_bass_kernel_spmd

F32 = mybir.dt.float32
BF16 = mybir.dt.bfloat16
OP = mybir.AluOpType
AF = mybir.ActivationFunctionType

H = 2048
V = 10
D4 = 512
B = 4
S = 2048
N_CORES = 8
T = (B * S) // N_CORES          # 1024 tokens per core
NCH = H // 128                  # 16 h-chunks
TB = T // 512                   # 2 token halves of 512
LN_EPS = 1e-5
N_ITERS = 5
# chunks evicted before ccol is ready take a two-step epilogue (ACT +ccol)
FOLD_J = 8

_PROG = None


def build_program():
    nc = bacc.Bacc("TRN2", target_bir_lowering=False, debug=False,
                   num_devices=N_CORES)

    xbf_d = nc.dram_tensor("xbfT", [128, NCH, T], BF16, kind="ExternalInput").ap()
    w1_d = nc.dram_tensor("w1t", [128, NCH, NCH * 128], BF16,
                          kind="ExternalInput").ap()
    wve_d = nc.dram_tensor("wve_t", [128, NCH, V], BF16, kind="ExternalInput").ap()
    w2t_d = nc.dram_tensor("w2t", [V, H], F32, kind="ExternalInput").ap()
    rve_d = nc.dram_tensor("rve_col", [V, 1], F32, kind="ExternalInput").ap()
    bve_d = nc.dram_tensor("bve_col", [V, 1], F32, kind="ExternalInput").ap()
    hasp_d = nc.dram_tensor("hasp_col", [V, 1], F32, kind="ExternalInput").ap()
    icn_d = nc.dram_tensor("icn_col", [V, 1], F32, kind="ExternalInput").ap()
    pft_d = nc.dram_tensor("pft", [V, V], F32, kind="ExternalInput").ap()
    var_d = nc.dram_tensor("var_bf", [V, D4], BF16, kind="ExternalInput").ap()
    cpt_d = nc.dram_tensor("cpt", [V, D4], F32, kind="ExternalInput").ap()
    nr1_d = nc.dram_tensor("neg_r1", [128, NCH], F32, kind="ExternalInput").ap()
    bout_d = nc.dram_tensor("bout_col", [128, NCH], F32, kind="ExternalInput").ap()
    out_d = nc.dram_tensor("outT", [H, T], F32, kind="ExternalOutput").ap()

    with tile.TileContext(nc) as tc:
        with (
            tc.tile_pool(name="pc", bufs=1) as pc,
            tc.tile_pool(name="pxb", bufs=16) as pxb,
            tc.tile_pool(name="pw1", bufs=16) as pw1,
            tc.tile_pool(name="px2", bufs=3) as px2,
            tc.tile_pool(name="pst", bufs=4) as pst,
            tc.tile_pool(name="psum", bufs=3, space="PSUM") as psum,
            tc.tile_pool(name="psx", bufs=4, space="PSUM") as psx,
            tc.tile_pool(name="pdram", bufs=1, space="DRAM") as pdram,
        ):
            # ---- small constants (gpsimd ring, separate from bulk DMA) ----
            wve_sb = pc.tile([128, NCH, V], BF16)
            nc.gpsimd.dma_start(out=wve_sb[:], in_=wve_d[:])
            w2t_sb = pc.tile([V, H], F32)
            nc.gpsimd.dma_start(out=w2t_sb[:], in_=w2t_d[:])
            rve_sb = pc.tile([V, 1], F32)
            nc.gpsimd.dma_start(out=rve_sb[:], in_=rve_d[:])
            bve_sb = pc.tile([V, 1], F32)
            nc.gpsimd.dma_start(out=bve_sb[:], in_=bve_d[:])
            hasp_sb = pc.tile([V, 1], F32)
            nc.gpsimd.dma_start(out=hasp_sb[:], in_=hasp_d[:])
            icn_sb = pc.tile([V, 1], F32)
            nc.gpsimd.dma_start(out=icn_sb[:], in_=icn_d[:])
            pft_sb = pc.tile([V, V], F32)
            nc.gpsimd.dma_start(out=pft_sb[:], in_=pft_d[:])
            var_sb = pc.tile([V, D4], BF16)
            nc.gpsimd.dma_start(out=var_sb[:], in_=var_d[:])
            cpt_sb = pc.tile([V, D4], F32)
            nc.gpsimd.dma_start(out=cpt_sb[:], in_=cpt_d[:])
            nr1_sb = pc.tile([128, NCH], F32)
            nc.gpsimd.dma_start(out=nr1_sb[:], in_=nr1_d[:])
            bout_sb = pc.tile([128, NCH], F32)
            nc.gpsimd.dma_start(out=bout_sb[:], in_=bout_d[:])
            ones_sb = pc.tile([128, 128], BF16)
            nc.vector.memset(ones_sb[:], 1.0)
            eps_ln = pc.tile([128, 1], F32)
            nc.vector.memset(eps_ln[:], LN_EPS)
            eps_pn = pc.tile([V, 1], F32)
            nc.vector.memset(eps_pn[:], 1e-16)

            # ---- PE warm-up: trip the HAM to full clock before real work ----
            warm_ps = psx.tile([128, 128], F32, tag="st", bufs=4, name="warm")
            for i in range(24):
                nc.tensor.matmul(warm_ps[:], ones_sb[:], ones_sb[:],
                                 start=True, stop=True)

            # ---- bulk DMAs ----
            # A dma_start trigger costs ~0.7us on the issuing engine, so the
            # trigger COUNT and issue-engine choice dominate arrival times:
            # xbf even chunks trigger from sync, odd from scalar (idle
            # early); split counts match the stats-phase consumption rate.
            # w1 triggers ride the gpsimd ring after the consts.
            xbfs = []
            for j in range(NCH):
                xbf = pxb.tile([128, T], BF16, tag="xbf", bufs=16, name=f"xbf{j}")
                xbfs.append(xbf)
            for j in range(NCH):
                eng = nc.sync if j % 2 == 0 else nc.scalar
                nsp = 4 if j < 2 else (2 if j < 8 else 1)
                for q in range(nsp):
                    p0 = q * (128 // nsp)
                    p1 = p0 + (128 // nsp)
                    eng.dma_start(out=xbfs[j][p0:p1, :],
                                  in_=xbf_d[p0:p1, j, :])
            w1_tiles = []
            for j in range(NCH):
                wt = pw1.tile([128, NCH * 128], BF16, tag="w1", bufs=16,
                              name=f"w1_{j}")
                nsp = 2 if j < 2 else 1
                for q in range(nsp):
                    p0 = q * (128 // nsp)
                    p1 = p0 + (128 // nsp)
                    nc.gpsimd.dma_start(out=wt[p0:p1, :], in_=w1_d[p0:p1, j, :])
                w1_tiles.append(wt)

            # ---- evidence phase, one token-half at a time (PSUM fit) ----
            mu_bc = pc.tile([128, T], BF16)
            rstd_bc = pc.tile([128, T], BF16)
            murstd_bc = pc.tile([128, T], BF16)
            var_bc = pc.tile([128, T], F32)
            lnv_bc = pc.tile([128, T], F32)
            mrs_acc = pc.tile([128, 1], F32)
            ev_acc = pc.tile([V, TB], F32)
            lgs = []
            for t in range(TB):
                sl = slice(t * 512, (t + 1) * 512)
                lg_ps = psx.tile([V, 512], F32, tag="st", bufs=4, name=f"lg{t}")
                sx_ps = psx.tile([128, 512], F32, tag="st", bufs=4,
                                 name=f"sx{t}")
                sq_ps = psx.tile([128, 512], F32, tag="st", bufs=4,
                                 name=f"sq{t}")
                for j in range(NCH):
                    x2 = px2.tile([128, 512], BF16, tag="x2", bufs=3,
                                  name=f"x2_{t}_{j}")
                    nc.vector.tensor_mul(x2[:], xbfs[j][:, sl], xbfs[j][:, sl])
                    nc.tensor.matmul(lg_ps[:], wve_sb[:, j, :],
                                     xbfs[j][:, sl],
                                     start=(j == 0), stop=(j == NCH - 1))
                    nc.tensor.matmul(sx_ps[:], ones_sb[:], xbfs[j][:, sl],
                                     start=(j == 0), stop=(j == NCH - 1))
                    nc.tensor.matmul(sq_ps[:], ones_sb[:], x2[:],
                                     start=(j == 0), stop=(j == NCH - 1))
                # LN stats for this half, broadcast [128, 512] form
                nc.vector.tensor_scalar_mul(mu_bc[:, sl], sx_ps[:], 1.0 / H)
                mu2 = pst.tile([128, 512], BF16, tag="scr", bufs=4,
                               name=f"mu2_{t}")
                nc.vector.tensor_mul(mu2[:], mu_bc[:, sl], mu_bc[:, sl])
                nc.vector.scalar_tensor_tensor(
                    out=var_bc[:, sl], in0=sq_ps[:], scalar=1.0 / H,
                    in1=mu2[:], op0=OP.mult, op1=OP.subtract)
                lgs.append(lg_ps)

            # rstd = (var+eps)^-0.5 via exp(-0.5*ln(.)) on ScalarE (~1e-5
            # rel); ACT Rsqrt is blocked (accuracy), DVE reciprocal is
            # Newton-iterative and far too slow here.  One full-T pass so
            # each ACT table loads once.
            nc.scalar.activation(lnv_bc[:], var_bc[:], AF.Ln, bias=eps_ln[:])
            nc.scalar.activation(rstd_bc[:], lnv_bc[:], AF.Exp, bias=0.0,
                                 scale=-0.5)
            # evidence partial: sum_t lg[v,t] * rstd[t]
            for t in range(TB):
                sl = slice(t * 512, (t + 1) * 512)
                lg_scr = pc.tile([V, 512], F32, tag="lgs", bufs=2,
                                 name=f"lg_scr{t}")
                nc.vector.scalar_tensor_tensor(
                    out=lg_scr[:], in0=lgs[t][:], scalar=1.0,
                    in1=rstd_bc[0:V, sl], op0=OP.mult, op1=OP.mult,
                    accum_out=ev_acc[:, t:t + 1])

            # murstd broadcast (used by every evict) + its token sum (for ev)
            nc.vector.scalar_tensor_tensor(
                out=murstd_bc[:], in0=mu_bc[:], scalar=1.0, in1=rstd_bc[:],
                op0=OP.mult, op1=OP.mult, accum_out=mrs_acc[:])

            # ---- evidence combine + AllReduce ----
            rv = pc.tile([V, 1], F32)
            nc.vector.tensor_mul(rv[:], mrs_acc[0:V, :], rve_sb[:])
            ev_sb = pc.tile([V, 1], F32)
            nc.vector.tensor_add(ev_sb[:], ev_acc[:, 0:1], ev_acc[:, 1:2])
            nc.vector.tensor_sub(ev_sb[:], ev_sb[:], rv[:])

            cc_in = pdram.tile([V, 1], F32)
            cc_out = pdram.tile([V, 1], F32)
            nc.gpsimd.dma_start(out=cc_in[:], in_=ev_sb[:])
            nc.gpsimd.collective_compute(
                "AllReduce", OP.add,
                replica_groups=[[0, 1], [2, 3], [4, 5], [6, 7]],
                ins=[cc_in.opt()], outs=[cc_out.opt()])
            cc_sb = pc.tile([V, 1], F32)
            nc.gpsimd.dma_start(out=cc_sb[:], in_=cc_out[:])

            # ---- belief propagation (tiny, overlaps main matmul) ----
            SIG_C = (0.2499968877665068, -0.020805674064028827,
                     2.0168972875466143e-03, -1.499637664404622e-04)

            def emit_sigmoid_poly(out, x, tag):
                # sigmoid(x) for |x| <= 1.3 as an odd degree-7 polynomial on
                # DVE (max err 1.5e-6) - avoids ACT Sigmoid table loads.
                c1, c3, c5, c7 = SIG_C
                x2p = pc.tile([V, 1], F32, name=f"sx2_{tag}")
                nc.vector.tensor_mul(x2p[:], x[:], x[:])
                p = pc.tile([V, 1], F32, name=f"sp_{tag}")
                nc.vector.tensor_scalar(p[:], x2p[:], c7, c5, op0=OP.mult,
                                        op1=OP.add)
                nc.vector.tensor_mul(p[:], p[:], x2p[:])
                nc.vector.tensor_scalar(p[:], p[:], c3, None, op0=OP.add)
                nc.vector.tensor_mul(p[:], p[:], x2p[:])
                nc.vector.tensor_scalar(p[:], p[:], c1, None, op0=OP.add)
                nc.vector.tensor_mul(p[:], p[:], x[:])
                nc.vector.tensor_scalar(out[:], p[:], 0.5, None, op0=OP.add)

            bp = {}

            def emit_bp_pre():
                ev_arg = pc.tile([V, 1], F32)
                nc.vector.tensor_scalar_mul(ev_arg[:], cc_sb[:], 1.0 / S)
                nc.vector.tensor_add(ev_arg[:], ev_arg[:], bve_sb[:])
                ev0 = pc.tile([V, 1], F32)
                emit_sigmoid_poly(ev0, ev_arg, "ev")
                m1 = pc.tile([V, 1], F32)
                nc.vector.tensor_scalar(m1[:], ev0[:], 0.1, None, op0=OP.is_gt)
                mask = pc.tile([V, 1], F32)
                nc.vector.tensor_scalar(mask[:], ev0[:], 0.9, None,
                                        op0=OP.is_lt)
                nc.vector.tensor_mul(mask[:], mask[:], m1[:])
                nc.vector.tensor_scalar(mask[:], mask[:], hasp_sb[:, 0:1],
                                        None, op0=OP.mult)
                probs = pc.tile([V, 1], F32)
                nc.vector.tensor_copy(probs[:], ev0[:])
                bp.update(mask=mask, probs=probs)

            def emit_bp_iter(it):
                mask, probs = bp["mask"], bp["probs"]
                lhsT = pc.tile([V, V], BF16, name=f"lhsT{it}")
                nc.vector.tensor_scalar(lhsT[:], pft_sb[:], probs[:, 0:1],
                                        None, op0=OP.mult)
                pe_ps = psx.tile([V, 512], F32, tag="st", bufs=4,
                                 name=f"pe{it}")
                nc.tensor.matmul(pe_ps[:], lhsT[:], var_sb[:],
                                 start=True, stop=True)
                pe_sb = pc.tile([V, D4], F32, tag="bscr", bufs=4,
                                name=f"pe_sb{it}")
                nc.vector.tensor_copy(pe_sb[:], pe_ps[:])
                bscr = pc.tile([V, D4], F32, tag="bscr", bufs=4,
                               name=f"bscr{it}")
                dot = pc.tile([V, 1], F32, name=f"dot{it}")
                nc.vector.scalar_tensor_tensor(
                    out=bscr[:], in0=pe_sb[:], scalar=1.0, in1=cpt_sb[:],
                    op0=OP.mult, op1=OP.mult, accum_out=dot[:])
                bscr2 = pc.tile([V, D4], F32, tag="bscr", bufs=4,
                                name=f"bscr2{it}")
                sqa = pc.tile([V, 1], F32, name=f"sqa{it}")
                nc.vector.scalar_tensor_tensor(
                    out=bscr2[:], in0=pe_sb[:], scalar=1.0, in1=pe_sb[:],
                    op0=OP.mult, op1=OP.mult, accum_out=sqa[:])
                # +1e-16 keeps parentless rows (pe == 0) finite; they are
                # masked out of the update anyway (matches the 1e-8 clamp).
                nc.scalar.activation(sqa[:], sqa[:], AF.Sqrt, bias=eps_pn[:])
                ipn = pc.tile([V, 1], F32, name=f"ipn{it}")
                nc.vector.reciprocal(ipn[:], sqa[:])
                s = pc.tile([V, 1], F32, name=f"s{it}")
                nc.vector.tensor_mul(s[:], dot[:], ipn[:])
                nc.vector.tensor_mul(s[:], s[:], icn_sb[:])
                cond = pc.tile([V, 1], F32, name=f"cond{it}")
                emit_sigmoid_poly(cond, s, f"it{it}")
                delta = pc.tile([V, 1], F32, name=f"delta{it}")
                nc.vector.tensor_sub(delta[:], cond[:], probs[:])
                nc.vector.tensor_mul(delta[:], delta[:], mask[:])
                nc.vector.tensor_add(probs[:], probs[:], delta[:])

            def emit_ccol():
                probs = bp["probs"]
                ccol_ps = psx.tile([128, 512], F32, tag="st", bufs=4,
                                   name="ccol_ps")
                for c in range(NCH):
                    nc.tensor.matmul(ccol_ps[:, c:c + 1],
                                     w2t_sb[:, c * 128:(c + 1) * 128],
                                     probs[:], start=True, stop=True)
                ccol_sb = pc.tile([128, NCH], F32)
                nc.vector.tensor_add(ccol_sb[:], ccol_ps[:, 0:NCH], bout_sb[:])
                bp["ccol"] = ccol_sb

            # ---- main matmul stream + per-chunk evict ----
            emit_bp_pre()
            pend = {}

            def emit_evict_half(j, t, acc, stage):
                # s3 = acc*rstd ; s4 = murstd*(-r1_j) + s3  (half-chunk ops
                # keep DVE latency low so bp/evict interleave tightly)
                sl = slice(t * 512, (t + 1) * 512)
                s3 = pst.tile([128, 512], BF16, tag="s3", bufs=3,
                              name=f"s3_{j}_{t}")
                nc.vector.scalar_tensor_tensor(
                    out=s3[:], in0=acc[:], scalar=1.0, in1=rstd_bc[:, sl],
                    op0=OP.mult, op1=OP.mult)
                s4 = pst.tile([128, 512], BF16, tag="s4", bufs=3,
                              name=f"s4_{j}_{t}")
                nc.vector.scalar_tensor_tensor(
                    out=s4[:], in0=murstd_bc[:, sl],
                    scalar=nr1_sb[:, j:j + 1],
                    in1=s3[:], op0=OP.mult, op1=OP.add)
                if j >= FOLD_J:
                    # + residual + ccol in one op, straight to DMA
                    nc.vector.scalar_tensor_tensor(
                        out=stage[:, sl], in0=xbfs[j][:, sl],
                        scalar=bp["ccol"][:, j:j + 1],
                        in1=s4[:], op0=OP.add, op1=OP.add)
                else:
                    # ccol not ready yet: + residual now, +ccol later on ACT
                    nc.vector.tensor_add(stage[:, sl], xbfs[j][:, sl], s4[:])

            def emit_late_out(j):
                stage = pend.pop(j)
                nc.scalar.activation(stage[:], stage[:], AF.Identity,
                                     bias=bp["ccol"][:, j:j + 1])
                nc.sync.dma_start(out=out_d[j * 128:(j + 1) * 128, :],
                                  in_=stage[:])

            for j in range(NCH):
                stage = pst.tile([128, T], F32, tag="stage", bufs=10,
                                 name=f"stage{j}")
                for t in range(TB):
                    sl = slice(t * 512, (t + 1) * 512)
                    acc = psum.tile([128, 512], F32, tag="acc", bufs=3,
                                    name=f"acc{j}_{t}")
                    for hin in range(NCH):
                        nc.tensor.matmul(
                            acc[:],
                            w1_tiles[j][:, hin * 128:(hin + 1) * 128],
                            xbfs[hin][:, sl],
                            start=(hin == 0), stop=(hin == NCH - 1))
                    emit_evict_half(j, t, acc, stage)
                if j < FOLD_J:
                    pend[j] = stage
                else:
                    nc.sync.dma_start(out=out_d[j * 128:(j + 1) * 128, :],
                                      in_=stage[:])
                if j == 2:
                    emit_bp_iter(0)
                    emit_bp_iter(1)
                if j == 3:
                    emit_bp_iter(2)
                    emit_bp_iter(3)
                if j == 4:
                    emit_bp_iter(4)
                    emit_ccol()
                if j == FOLD_J - 1:
                    for jj in range(FOLD_J):
                        emit_late_out(jj)

    nc.compile()
    return nc


def _host_prep(hidden_states, gamma, beta, W_ve, b_ve, var_emb, cpt_emb,
               W_out, b_out, parents):
    f32 = np.float32
    bf16 = ml_dtypes.bfloat16
    x = np.asarray(hidden_states, f32).reshape(B * S, H)
    gamma = np.asarray(gamma, f32)
    beta = np.asarray(beta, f32)
    W_ve = np.asarray(W_ve, f32)
    b_ve = np.asarray(b_ve, f32)
    var_emb = np.asarray(var_emb, f32)
    cpt_emb = np.asarray(cpt_emb, f32)
    W_out = np.asarray(W_out, f32)
    b_out = np.asarray(b_out, f32)
    parents = np.asarray(parents)

    W1 = W_out[:, :H]
    W1g = W1 * gamma[None, :]
    # stationary blocks: w1t[p, hin, j*128+c] = W1g[j*128+c, hin*128+p]
    w1t = np.ascontiguousarray(
        W1g.T.reshape(NCH, 128, NCH, 128).transpose(1, 2, 0, 3)
        .reshape(128, NCH, NCH * 128)).astype(bf16)
    w2t = np.ascontiguousarray(W_out[:, H:].T)
    Wveg = W_ve * gamma[None, :]
    wve_t = np.ascontiguousarray(
        Wveg.T.reshape(NCH, 128, V).transpose(1, 0, 2)).astype(bf16)
    rve_col = Wveg.astype(bf16).astype(f32).sum(axis=1).reshape(V, 1).astype(f32)
    bve_col = (b_ve + W_ve @ beta).reshape(V, 1).astype(f32)
    hasp_col = (parents.sum(axis=1) > 0).astype(f32).reshape(V, 1)
    icn_col = (1.0 / np.maximum(
        np.sqrt((cpt_emb * cpt_emb).sum(axis=1)), 1e-8)).reshape(V, 1).astype(f32)
    pft = np.ascontiguousarray(parents.T.astype(f32))
    var_bf = var_emb.astype(bf16)
    cpt = np.ascontiguousarray(cpt_emb, f32)
    neg_r1 = np.ascontiguousarray((-W1g.sum(axis=1)).reshape(NCH, 128).T, f32)
    bout_col = np.ascontiguousarray(
        (b_out + W1 @ beta).reshape(NCH, 128).T, f32)

    shared = dict(w1t=w1t, w2t=w2t, wve_t=wve_t, rve_col=rve_col,
                  bve_col=bve_col, hasp_col=hasp_col, icn_col=icn_col,
                  pft=pft, var_bf=var_bf, cpt=cpt, neg_r1=neg_r1,
                  bout_col=bout_col)
    in_maps = []
    for c in range(N_CORES):
        xT = x[c * T:(c + 1) * T, :].T.astype(bf16)       # [H, T]
        xbfT = np.ascontiguousarray(
            xT.reshape(NCH, 128, T).transpose(1, 0, 2))   # [128, NCH, T]
        in_maps.append(dict(shared, xbfT=xbfT))
    return in_maps


def kernel(**inputs):
    global _PROG
    if _PROG is None:
        _PROG = build_program()
    nc = _PROG
    in_maps = _host_prep(**inputs)
    res = run_bass_kernel_spmd(nc, in_maps, list(range(N_CORES)))
    out = np.empty((B * S, H), np.float32)
    for c in range(N_CORES):
        out[c * T:(c + 1) * T, :] = res.results[c]["outT"].T
    return out.reshape(B, S, H)


# revision 22
# speedup vs baseline: 1.1069x; 1.1069x over previous
"""Trainium2 Bass kernel for the BayesianBeliefNetwork block (8-core SPMD).

Math (see problem reference):
  h    = LayerNorm(x)*gamma + beta                          [B,S,H]
  ev   = sigmoid(mean_s(h @ W_ve.T + b_ve))                 [B,V]
  post = belief-prop(ev, parents, var_emb, cpt_emb)         [B,V]  (5 iters)
  out  = [h, post] @ W_out.T + b_out + x                    [B,S,H]

Sharding: data-parallel over the B*S = 8192 tokens; core c owns 1024 tokens
(batch b = c//2, sequence half c%2).  Parameters replicated.  The per-batch
evidence is completed with a pairwise AllReduce of the [V] partial logit
sums between the two cores sharing a batch.

Device layout: transposed - H on partitions, tokens on the free axis.
LayerNorm folds into the matmul epilogue:

  out^T[ho,t] = rstd_t * (W1g^T xbf)[ho,t]        W1g = W_out[:, :H]*gamma
              - r1[ho] * (mu_t*rstd_t)            r1  = W1g.sum(hin)
              + xbf^T[ho,t]                       (bf16 residual)
              + (b_out + W1@beta + W2@post)[ho]   (after belief prop)

Schedule: evidence path FIRST (logits + sum(x) + sum(x^2) ones-stationary
matmuls over all chunks, ~25us), so the tiny AllReduce issues early and
belief-prop + ccol complete while the dominant ~131us W1 matmul stream is
still running; each output chunk is evicted and DMA'd as soon as its
accumulation group (and ccol) is done, leaving a ~4us tail after the last
matmul.  PE warm-up matmuls trip the HAM clock gate before real work.

rstd = Rsqrt(var+eps) on ScalarE (a single activation table, also reused
by belief-prop's 1/||pe||).  The cosine's 1/denom parent normalization
cancels (scale-invariant) so belief prop needs no parent-count division;
1/||cpt|| is precomputed on the host.
"""

import numpy as np
import ml_dtypes

import concourse.bass as bass
import concourse.tile as tile
from concourse import bacc, mybir
from concourse.bass_utils import run_bass_kernel_spmd

F32 = mybir.dt.float32
BF16 = mybir.dt.bfloat16
OP = mybir.AluOpType
AF = mybir.ActivationFunctionType

H = 2048
V = 10
D4 = 512
B = 4
S = 2048
N_CORES = 8
T = (B * S) // N_CORES          # 1024 tokens per core
NCH = H // 128                  # 16 h-chunks
TB = T // 512                   # 2 token halves of 512
LN_EPS = 1e-5
N_ITERS = 5
# chunks evicted before ccol is ready take a two-step epilogue (ACT +ccol)
FOLD_J = 9

_PROG = None


def build_program():
    nc = bacc.Bacc("TRN2", target_bir_lowering=False, debug=False,
                   num_devices=N_CORES)

    xbf_d = nc.dram_tensor("xbfT", [128, NCH, T], BF16, kind="ExternalInput").ap()
    w1_d = nc.dram_tensor("w1t", [128, NCH, NCH * 128], BF16,
                          kind="ExternalInput").ap()
    wve_d = nc.dram_tensor("wve_t", [128, NCH, V], BF16, kind="ExternalInput").ap()
    w2t_d = nc.dram_tensor("w2t", [V, H], F32, kind="ExternalInput").ap()
    rve_d = nc.dram_tensor("rve_col", [V, 1], F32, kind="ExternalInput").ap()
    bve_d = nc.dram_tensor("bve_col", [V, 1], F32, kind="ExternalInput").ap()
    hasp_d = nc.dram_tensor("hasp_col", [V, 1], F32, kind="ExternalInput").ap()
    icn_d = nc.dram_tensor("icn_col", [V, 1], F32, kind="ExternalInput").ap()
    pft_d = nc.dram_tensor("pft", [V, V], F32, kind="ExternalInput").ap()
    var_d = nc.dram_tensor("var_bf", [V, D4], BF16, kind="ExternalInput").ap()
    cpt_d = nc.dram_tensor("cpt", [V, D4], F32, kind="ExternalInput").ap()
    nr1_d = nc.dram_tensor("neg_r1", [128, NCH], F32, kind="ExternalInput").ap()
    bout_d = nc.dram_tensor("bout_col", [128, NCH], F32, kind="ExternalInput").ap()
    out_d = nc.dram_tensor("outT", [H, T], F32, kind="ExternalOutput").ap()

    with tile.TileContext(nc) as tc:
        with (
            tc.tile_pool(name="pc", bufs=1) as pc,
            tc.tile_pool(name="pxb", bufs=16) as pxb,
            tc.tile_pool(name="pw1", bufs=16) as pw1,
            tc.tile_pool(name="px2", bufs=3) as px2,
            tc.tile_pool(name="pst", bufs=4) as pst,
            tc.tile_pool(name="psum", bufs=2, space="PSUM") as psum,
            tc.tile_pool(name="psx", bufs=6, space="PSUM") as psx,
            tc.tile_pool(name="pdram", bufs=1, space="DRAM") as pdram,
        ):
            # ---- small constants (gpsimd ring, separate from bulk DMA) ----
            wve_sb = pc.tile([128, NCH, V], BF16)
            nc.gpsimd.dma_start(out=wve_sb[:], in_=wve_d[:])
            w2t_sb = pc.tile([V, H], F32)
            nc.gpsimd.dma_start(out=w2t_sb[:], in_=w2t_d[:])
            rve_sb = pc.tile([V, 1], F32)
            nc.gpsimd.dma_start(out=rve_sb[:], in_=rve_d[:])
            bve_sb = pc.tile([V, 1], F32)
            nc.gpsimd.dma_start(out=bve_sb[:], in_=bve_d[:])
            hasp_sb = pc.tile([V, 1], F32)
            nc.gpsimd.dma_start(out=hasp_sb[:], in_=hasp_d[:])
            icn_sb = pc.tile([V, 1], F32)
            nc.gpsimd.dma_start(out=icn_sb[:], in_=icn_d[:])
            pft_sb = pc.tile([V, V], F32)
            nc.gpsimd.dma_start(out=pft_sb[:], in_=pft_d[:])
            var_sb = pc.tile([V, D4], BF16)
            nc.gpsimd.dma_start(out=var_sb[:], in_=var_d[:])
            cpt_sb = pc.tile([V, D4], F32)
            nc.gpsimd.dma_start(out=cpt_sb[:], in_=cpt_d[:])
            nr1_sb = pc.tile([128, NCH], F32)
            nc.gpsimd.dma_start(out=nr1_sb[:], in_=nr1_d[:])
            bout_sb = pc.tile([128, NCH], F32)
            nc.gpsimd.dma_start(out=bout_sb[:], in_=bout_d[:])
            ones_sb = pc.tile([128, 128], BF16)
            nc.vector.memset(ones_sb[:], 1.0)
            eps_ln = pc.tile([128, 1], F32)
            nc.vector.memset(eps_ln[:], LN_EPS)
            eps_pn = pc.tile([V, 1], F32)
            nc.vector.memset(eps_pn[:], 1e-16)

            # ---- PE warm-up: trip the HAM to full clock before real work ----
            warm_ps = psx.tile([128, 128], F32, tag="st", bufs=6, name="warm")
            for i in range(24):
                nc.tensor.matmul(warm_ps[:], ones_sb[:], ones_sb[:],
                                 start=True, stop=True)

            # ---- bulk DMAs ----
            # A dma_start trigger costs ~0.7us on the issuing engine, so the
            # trigger COUNT and issue-engine choice dominate arrival times:
            # xbf even chunks trigger from sync, odd from scalar (idle
            # early); split counts match the stats-phase consumption rate.
            # w1 triggers ride the gpsimd ring after the consts.
            xbfs = []
            for j in range(NCH):
                xbf = pxb.tile([128, T], BF16, tag="xbf", bufs=16, name=f"xbf{j}")
                xbfs.append(xbf)
            for j in range(NCH):
                eng = nc.sync if j % 2 == 0 else nc.scalar
                nsp = 4 if j < 2 else (2 if j < 8 else 1)
                for q in range(nsp):
                    p0 = q * (128 // nsp)
                    p1 = p0 + (128 // nsp)
                    eng.dma_start(out=xbfs[j][p0:p1, :],
                                  in_=xbf_d[p0:p1, j, :])
            w1_tiles = []
            for j in range(NCH):
                wt = pw1.tile([128, NCH * 128], BF16, tag="w1", bufs=16,
                              name=f"w1_{j}")
                nsp = 2 if j < 2 else 1
                for q in range(nsp):
                    p0 = q * (128 // nsp)
                    p1 = p0 + (128 // nsp)
                    nc.gpsimd.dma_start(out=wt[p0:p1, :], in_=w1_d[p0:p1, j, :])
                w1_tiles.append(wt)

            # ---- evidence phase: both token-halves interleaved per chunk ----
            mu_bc = pc.tile([128, T], BF16)
            rstd_bc = pc.tile([128, T], BF16)
            murstd_bc = pc.tile([128, T], BF16)
            var_bc = pc.tile([128, T], F32)
            lnv_bc = pc.tile([128, T], F32)
            mrs_acc = pc.tile([128, 1], F32)
            ev_acc = pc.tile([V, TB], F32)
            lgs = [psx.tile([V, 512], F32, tag="st", bufs=6, name=f"lg{t}")
                   for t in range(TB)]
            sxs = [psx.tile([128, 512], F32, tag="st", bufs=6, name=f"sx{t}")
                   for t in range(TB)]
            sqs = [psx.tile([128, 512], F32, tag="st", bufs=6, name=f"sq{t}")
                   for t in range(TB)]
            for j in range(NCH):
                x2 = px2.tile([128, T], BF16, tag="x2", bufs=3,
                              name=f"x2_{j}")
                nc.vector.tensor_mul(x2[:], xbfs[j][:], xbfs[j][:])
                for t in range(TB):
                    sl = slice(t * 512, (t + 1) * 512)
                    nc.tensor.matmul(lgs[t][:], wve_sb[:, j, :],
                                     xbfs[j][:, sl],
                                     start=(j == 0), stop=(j == NCH - 1))
                    nc.tensor.matmul(sxs[t][:], ones_sb[:], xbfs[j][:, sl],
                                     start=(j == 0), stop=(j == NCH - 1))
                    nc.tensor.matmul(sqs[t][:], ones_sb[:], x2[:, sl],
                                     start=(j == 0), stop=(j == NCH - 1))
            for t in range(TB):
                sl = slice(t * 512, (t + 1) * 512)
                # LN stats for this half, broadcast [128, 512] form
                nc.vector.tensor_scalar_mul(mu_bc[:, sl], sxs[t][:], 1.0 / H)
                mu2 = pst.tile([128, 512], BF16, tag="scr", bufs=4,
                               name=f"mu2_{t}")
                nc.vector.tensor_mul(mu2[:], mu_bc[:, sl], mu_bc[:, sl])
                nc.vector.scalar_tensor_tensor(
                    out=var_bc[:, sl], in0=sqs[t][:], scalar=1.0 / H,
                    in1=mu2[:], op0=OP.mult, op1=OP.subtract)

            # rstd = (var+eps)^-0.5 via exp(-0.5*ln(.)) on ScalarE (~1e-5
            # rel); ACT Rsqrt is blocked (accuracy), DVE reciprocal is
            # Newton-iterative and far too slow here.  One full-T pass so
            # each ACT table loads once.
            nc.scalar.activation(lnv_bc[:], var_bc[:], AF.Ln, bias=eps_ln[:])
            nc.scalar.activation(rstd_bc[:], lnv_bc[:], AF.Exp, bias=0.0,
                                 scale=-0.5)
            # evidence partial: sum_t lg[v,t] * rstd[t]
            for t in range(TB):
                sl = slice(t * 512, (t + 1) * 512)
                lg_scr = pc.tile([V, 512], F32, tag="lgs", bufs=2,
                                 name=f"lg_scr{t}")
                nc.vector.scalar_tensor_tensor(
                    out=lg_scr[:], in0=lgs[t][:], scalar=1.0,
                    in1=rstd_bc[0:V, sl], op0=OP.mult, op1=OP.mult,
                    accum_out=ev_acc[:, t:t + 1])

            # murstd broadcast (used by every evict) + its token sum (for ev)
            nc.vector.scalar_tensor_tensor(
                out=murstd_bc[:], in0=mu_bc[:], scalar=1.0, in1=rstd_bc[:],
                op0=OP.mult, op1=OP.mult, accum_out=mrs_acc[:])

            # ---- evidence combine + AllReduce ----
            rv = pc.tile([V, 1], F32)
            nc.vector.tensor_mul(rv[:], mrs_acc[0:V, :], rve_sb[:])
            ev_sb = pc.tile([V, 1], F32)
            nc.vector.tensor_add(ev_sb[:], ev_acc[:, 0:1], ev_acc[:, 1:2])
            nc.vector.tensor_sub(ev_sb[:], ev_sb[:], rv[:])

            cc_in = pdram.tile([V, 1], F32)
            cc_out = pdram.tile([V, 1], F32)
            nc.gpsimd.dma_start(out=cc_in[:], in_=ev_sb[:])
            nc.gpsimd.collective_compute(
                "AllReduce", OP.add,
                replica_groups=[[0, 1], [2, 3], [4, 5], [6, 7]],
                ins=[cc_in.opt()], outs=[cc_out.opt()])
            cc_sb = pc.tile([V, 1], F32)
            nc.gpsimd.dma_start(out=cc_sb[:], in_=cc_out[:])

            # ---- belief propagation (tiny, overlaps main matmul) ----
            SIG_C = (0.2499968877665068, -0.020805674064028827,
                     2.0168972875466143e-03, -1.499637664404622e-04)

            def emit_sigmoid_poly(out, x, tag):
                # sigmoid(x) for |x| <= 1.3 as an odd degree-7 polynomial on
                # DVE (max err 1.5e-6) - avoids ACT Sigmoid table loads.
                c1, c3, c5, c7 = SIG_C
                x2p = pc.tile([V, 1], F32, name=f"sx2_{tag}")
                nc.vector.tensor_mul(x2p[:], x[:], x[:])
                p = pc.tile([V, 1], F32, name=f"sp_{tag}")
                nc.vector.tensor_scalar(p[:], x2p[:], c7, c5, op0=OP.mult,
                                        op1=OP.add)
                nc.vector.tensor_mul(p[:], p[:], x2p[:])
                nc.vector.tensor_scalar(p[:], p[:], c3, None, op0=OP.add)
                nc.vector.tensor_mul(p[:], p[:], x2p[:])
                nc.vector.tensor_scalar(p[:], p[:], c1, None, op0=OP.add)
                nc.vector.tensor_mul(p[:], p[:], x[:])
                nc.vector.tensor_scalar(out[:], p[:], 0.5, None, op0=OP.add)

            bp = {}

            def emit_bp_pre():
                ev_arg = pc.tile([V, 1], F32)
                nc.vector.tensor_scalar_mul(ev_arg[:], cc_sb[:], 1.0 / S)
                nc.vector.tensor_add(ev_arg[:], ev_arg[:], bve_sb[:])
                ev0 = pc.tile([V, 1], F32)
                emit_sigmoid_poly(ev0, ev_arg, "ev")
                m1 = pc.tile([V, 1], F32)
                nc.vector.tensor_scalar(m1[:], ev0[:], 0.1, None, op0=OP.is_gt)
                mask = pc.tile([V, 1], F32)
                nc.vector.tensor_scalar(mask[:], ev0[:], 0.9, None,
                                        op0=OP.is_lt)
                nc.vector.tensor_mul(mask[:], mask[:], m1[:])
                nc.vector.tensor_scalar(mask[:], mask[:], hasp_sb[:, 0:1],
                                        None, op0=OP.mult)
                probs = pc.tile([V, 1], F32)
                nc.vector.tensor_copy(probs[:], ev0[:])
                bp.update(mask=mask, probs=probs)

            def emit_bp_iter(it):
                mask, probs = bp["mask"], bp["probs"]
                lhsT = pc.tile([V, V], BF16, name=f"lhsT{it}")
                nc.vector.tensor_scalar(lhsT[:], pft_sb[:], probs[:, 0:1],
                                        None, op0=OP.mult)
                pe_ps = psx.tile([V, 512], F32, tag="st", bufs=6,
                                 name=f"pe{it}")
                nc.tensor.matmul(pe_ps[:], lhsT[:], var_sb[:],
                                 start=True, stop=True)
                pe_sb = pc.tile([V, D4], F32, tag="bscr", bufs=4,
                                name=f"pe_sb{it}")
                nc.vector.tensor_copy(pe_sb[:], pe_ps[:])
                bscr = pc.tile([V, D4], F32, tag="bscr", bufs=4,
                               name=f"bscr{it}")
                dot = pc.tile([V, 1], F32, name=f"dot{it}")
                nc.vector.scalar_tensor_tensor(
                    out=bscr[:], in0=pe_sb[:], scalar=1.0, in1=cpt_sb[:],
                    op0=OP.mult, op1=OP.mult, accum_out=dot[:])
                bscr2 = pc.tile([V, D4], F32, tag="bscr", bufs=4,
                                name=f"bscr2{it}")
                sqa = pc.tile([V, 1], F32, name=f"sqa{it}")
                nc.vector.scalar_tensor_tensor(
                    out=bscr2[:], in0=pe_sb[:], scalar=1.0, in1=pe_sb[:],
                    op0=OP.mult, op1=OP.mult, accum_out=sqa[:])
                # +1e-16 keeps parentless rows (pe == 0) finite; they are
                # masked out of the update anyway (matches the 1e-8 clamp).
                nc.scalar.activation(sqa[:], sqa[:], AF.Sqrt, bias=eps_pn[:])
                ipn = pc.tile([V, 1], F32, name=f"ipn{it}")
                nc.vector.reciprocal(ipn[:], sqa[:])
                s = pc.tile([V, 1], F32, name=f"s{it}")
                nc.vector.tensor_mul(s[:], dot[:], ipn[:])
                nc.vector.tensor_mul(s[:], s[:], icn_sb[:])
                cond = pc.tile([V, 1], F32, name=f"cond{it}")
                emit_sigmoid_poly(cond, s, f"it{it}")
                delta = pc.tile([V, 1], F32, name=f"delta{it}")
                nc.vector.tensor_sub(delta[:], cond[:], probs[:])
                nc.vector.tensor_mul(delta[:], delta[:], mask[:])
                nc.vector.tensor_add(probs[:], probs[:], delta[:])

            def emit_ccol():
                probs = bp["probs"]
                ccol_ps = psx.tile([128, 512], F32, tag="st", bufs=6,
                                   name="ccol_ps")
                for c in range(NCH):
                    nc.tensor.matmul(ccol_ps[:, c:c + 1],
                                     w2t_sb[:, c * 128:(c + 1) * 128],
                                     probs[:], start=True, stop=True)
                ccol_sb = pc.tile([128, NCH], F32)
                nc.vector.tensor_add(ccol_sb[:], ccol_ps[:, 0:NCH], bout_sb[:])
                bp["ccol"] = ccol_sb

            # ---- main matmul stream + per-chunk evict ----
            emit_bp_pre()
            pend = {}

            def emit_evict_half(j, t, acc, stage):
                # s3 = acc*rstd ; s4 = murstd*(-r1_j) + s3  (half-chunk ops
                # keep DVE latency low so bp/evict interleave tightly)
                sl = slice(t * 512, (t + 1) * 512)
                s3 = pst.tile([128, 512], BF16, tag="s3", bufs=3,
                              name=f"s3_{j}_{t}")
                nc.vector.scalar_tensor_tensor(
                    out=s3[:], in0=acc[:], scalar=1.0, in1=rstd_bc[:, sl],
                    op0=OP.mult, op1=OP.mult)
                s4 = pst.tile([128, 512], BF16, tag="s4", bufs=3,
                              name=f"s4_{j}_{t}")
                nc.vector.scalar_tensor_tensor(
                    out=s4[:], in0=murstd_bc[:, sl],
                    scalar=nr1_sb[:, j:j + 1],
                    in1=s3[:], op0=OP.mult, op1=OP.add)
                if j >= FOLD_J:
                    # + residual + ccol in one op, straight to DMA
                    nc.vector.scalar_tensor_tensor(
                        out=stage[:, sl], in0=xbfs[j][:, sl],
                        scalar=bp["ccol"][:, j:j + 1],
                        in1=s4[:], op0=OP.add, op1=OP.add)
                else:
                    # ccol not ready yet: + residual now, +ccol later on ACT
                    nc.vector.tensor_add(stage[:, sl], xbfs[j][:, sl], s4[:])

            def emit_out_dma(j, stage):
                # partition-split so the last chunks drain multiple queues
                nsp = 4 if j >= NCH - 2 else 2
                for q in range(nsp):
                    p0 = q * (128 // nsp)
                    p1 = p0 + (128 // nsp)
                    nc.sync.dma_start(
                        out=out_d[j * 128 + p0:j * 128 + p1, :],
                        in_=stage[p0:p1, :])

            def emit_late_out(j):
                stage = pend.pop(j)
                nc.scalar.activation(stage[:], stage[:], AF.Identity,
                                     bias=bp["ccol"][:, j:j + 1])
                emit_out_dma(j, stage)

            for j in range(NCH):
                stage = pst.tile([128, T], F32, tag="stage", bufs=10,
                                 name=f"stage{j}")
                for t in range(TB):
                    sl = slice(t * 512, (t + 1) * 512)
                    acc = psum.tile([128, 512], F32, tag="acc", bufs=2,
                                    name=f"acc{j}_{t}")
                    for hin in range(NCH):
                        nc.tensor.matmul(
                            acc[:],
                            w1_tiles[j][:, hin * 128:(hin + 1) * 128],
                            xbfs[hin][:, sl],
                            start=(hin == 0), stop=(hin == NCH - 1))
                    emit_evict_half(j, t, acc, stage)
                if j < FOLD_J:
                    pend[j] = stage
                else:
                    emit_out_dma(j, stage)
                if j == 4:
                    emit_bp_iter(0)
                    emit_bp_iter(1)
                if j == 5:
                    emit_bp_iter(2)
                    emit_bp_iter(3)
                if j == 6:
                    emit_bp_iter(4)
                    emit_ccol()
                if j == FOLD_J - 1:
                    for jj in range(FOLD_J):
                        emit_late_out(jj)

    nc.compile()
    return nc


def _host_prep(hidden_states, gamma, beta, W_ve, b_ve, var_emb, cpt_emb,
               W_out, b_out, parents):
    f32 = np.float32
    bf16 = ml_dtypes.bfloat16
    x = np.asarray(hidden_states, f32).reshape(B * S, H)
    gamma = np.asarray(gamma, f32)
    beta = np.asarray(beta, f32)
    W_ve = np.asarray(W_ve, f32)
    b_ve = np.asarray(b_ve, f32)
    var_emb = np.asarray(var_emb, f32)
    cpt_emb = np.asarray(cpt_emb, f32)
    W_out = np.asarray(W_out, f32)
    b_out = np.asarray(b_out, f32)
    parents = np.asarray(parents)

    W1 = W_out[:, :H]
    W1g = W1 * gamma[None, :]
    # stationary blocks: w1t[p, hin, j*128+c] = W1g[j*128+c, hin*128+p]
    w1t = np.ascontiguousarray(
        W1g.T.reshape(NCH, 128, NCH, 128).transpose(1, 2, 0, 3)
        .reshape(128, NCH, NCH * 128)).astype(bf16)
    w2t = np.ascontiguousarray(W_out[:, H:].T)
    Wveg = W_ve * gamma[None, :]
    wve_t = np.ascontiguousarray(
        Wveg.T.reshape(NCH, 128, V).transpose(1, 0, 2)).astype(bf16)
    rve_col = Wveg.astype(bf16).astype(f32).sum(axis=1).reshape(V, 1).astype(f32)
    bve_col = (b_ve + W_ve @ beta).reshape(V, 1).astype(f32)
    hasp_col = (parents.sum(axis=1) > 0).astype(f32).reshape(V, 1)
    icn_col = (1.0 / np.maximum(
        np.sqrt((cpt_emb * cpt_emb).sum(axis=1)), 1e-8)).reshape(V, 1).astype(f32)
    pft = np.ascontiguousarray(parents.T.astype(f32))
    var_bf = var_emb.astype(bf16)
    cpt = np.ascontiguousarray(cpt_emb, f32)
    neg_r1 = np.ascontiguousarray((-W1g.sum(axis=1)).reshape(NCH, 128).T, f32)
    bout_col = np.ascontiguousarray(
        (b_out + W1 @ beta).reshape(NCH, 128).T, f32)

    shared = dict(w1t=w1t, w2t=w2t, wve_t=wve_t, rve_col=rve_col,
                  bve_col=bve_col, hasp_col=hasp_col, icn_col=icn_col,
                  pft=pft, var_bf=var_bf, cpt=cpt, neg_r1=neg_r1,
                  bout_col=bout_col)
    in_maps = []
    for c in range(N_CORES):
        xT = x[c * T:(c + 1) * T, :].T.astype(bf16)       # [H, T]
        xbfT = np.ascontiguousarray(
            xT.reshape(NCH, 128, T).transpose(1, 0, 2))   # [128, NCH, T]
        in_maps.append(dict(shared, xbfT=xbfT))
    return in_maps


def kernel(**inputs):
    global _PROG
    if _PROG is None:
        _PROG = build_program()
    nc = _PROG
    in_maps = _host_prep(**inputs)
    res = run_bass_kernel_spmd(nc, in_maps, list(range(N_CORES)))
    out = np.empty((B * S, H), np.float32)
    for c in range(N_CORES):
        out[c * T:(c + 1) * T, :] = res.results[c]["outT"].T
    return out.reshape(B, S, H)
